# revision 8
# baseline (speedup 1.0000x reference)
"""Trainium2 Bass kernel for nn_LocalNet (binary-tree reduction network).

Computation: x [2048, 65536] f32; 16 levels of per-pair Linear(2,1) + ReLU
(no ReLU on the last level) -> out [2048, 1].

Strategy (pure data parallel, 8 cores, 256 rows each):
- Host: within each 512-feature partition block, permute columns by 9-bit
  bit-reversal.  This makes every tree level "planar": pair partners sit at
  (i, i + n/2), so all DVE accesses are unit-stride and fp16 tensor_tensor
  runs in 2x mode.  x is pre-cast to fp16 on host (bit-identical to the
  on-chip ScalarE cast it replaces; compute is fp16 anyway) halving DMA.
- Device, per core: stream groups of R rows as [128, R, 512] fp16 tiles
  (partition p holds that row's features [512p, 512p+512) bitrev-permuted),
  then levels 0..8:
      z  = s * wcat             (VectorE; wcat = [W0 | W1] planar, row-bcast)
      v  = z[:half] + z[half:]  (GpSimd for levels 0-1, else VectorE)
      s' = relu(v)              (ScalarE)
  Level-8 outputs accumulate into a [128, 256] staging tile (node q of each
  row's 128-node level-9 input lives on partition q).  Two 128x128 DMA-xbar
  transposes flip rows onto partitions; levels 9..15 then run along the free
  axis with host-replicated weights.  Final [256,1] f32 DMAed out per core.
"""

import sys

for _p in ("/opt/trn_rl_repo",):
    if _p not in sys.path:
        sys.path.insert(0, _p)

import numpy as np

TREE_DEPTH = 16
BATCH = 2048
FEATS = 65536
NCORES = 8
ROWS = BATCH // NCORES      # 256 rows per core
P = 128                     # SBUF partitions
SUB = FEATS // P            # 512 features per partition subtree
R = 16                      # rows per streamed group
G = ROWS // R               # groups
IN_LEVELS = 9               # levels 0..8 run inside partitions
GPSIMD_ADD_LEVELS = 2       # adds for levels 0..k-1 go to GpSimd


def _bitrev_array(bits):
    n = 1 << bits
    r = np.zeros(n, dtype=np.int64)
    for i in range(n):
        v = 0
        for b in range(bits):
            if i & (1 << b):
                v |= 1 << (bits - 1 - b)
        r[i] = v
    return r


def _host_pack(x, weights):
    """Build per-core input arrays + shared weight arrays."""
    brev = _bitrev_array(9)
    xs = np.asarray(x, dtype=np.float32).reshape(BATCH, P, SUB)[:, :, brev]
    xs = xs.astype(np.float16)
    per_core_x = []
    for c in range(NCORES):
        xc = np.ascontiguousarray(xs[c * ROWS:(c + 1) * ROWS].transpose(1, 0, 2))
        per_core_x.append(xc)  # [128, 256, 512] fp16

    blocks = []
    for l in range(IN_LEVELS):
        n = SUB >> l
        half = n // 2
        idx = _bitrev_array(8 - l) if half > 1 else np.zeros(1, dtype=np.int64)
        W = np.asarray(weights[l])                      # [2^(15-l), 2]
        q = np.arange(P)[:, None]
        g = q * half + idx[None, :]
        W0 = W[g, 0].astype(np.float16)
        W1 = W[g, 1].astype(np.float16)
        blocks.append(np.concatenate([W0, W1], axis=1))  # [128, n] fp16
    for l in range(IN_LEVELS, TREE_DEPTH):
        m = 1 << (15 - l)
        W = np.asarray(weights[l]).astype(np.float16)    # [m, 2]
        blocks.append(np.broadcast_to(W[None, :, 0], (P, m)))
        blocks.append(np.broadcast_to(W[None, :, 1], (P, m)))
    wall = np.ascontiguousarray(np.concatenate(blocks, axis=1))  # [128, WTOT]
    wparams = {"wall": wall}
    return per_core_x, wparams


_CACHED = {}


def _build():
    """Build the (SPMD-identical) Bass module once."""
    if "nc" in _CACHED:
        return _CACHED["nc"]
    import concourse.bacc as bacc
    import concourse.mybir as mybir
    import concourse.tile as tile

    dt = mybir.dt
    nc = bacc.Bacc(
        "TRN2", target_bir_lowering=False, debug=False, num_devices=NCORES)

    xin = nc.declare_dram_parameter("x", [P, ROWS, SUB], dt.float16, isOutput=False)
    WTOT = sum(SUB >> l for l in range(IN_LEVELS)) + 2 * sum(
        1 << (15 - l) for l in range(IN_LEVELS, TREE_DEPTH))
    wallp = nc.declare_dram_parameter("wall", [P, WTOT], dt.float16, isOutput=False)
    out = nc.declare_dram_parameter("out", [ROWS, 1], dt.float32, isOutput=True)

    relu_t = mybir.ActivationFunctionType.Relu

    with tile.TileContext(nc) as tc:
        with (
            tc.tile_pool(name="sio", bufs=3) as spool,
            tc.tile_pool(name="early", bufs=2) as early,
            tc.tile_pool(name="deep", bufs=2) as deep,
            tc.tile_pool(name="weights", bufs=1) as wp,
            tc.tile_pool(name="stage", bufs=1) as stp,
        ):
            # Load all weights in ONE DMA; slice views per level.
            wtile = wp.tile([P, WTOT], dt.float16, tag="wall")
            nc.sync.dma_start(out=wtile[:], in_=wallp[:])
            wtiles = []
            off = 0
            for l in range(IN_LEVELS):
                n = SUB >> l
                wtiles.append(wtile[:, off:off + n])
                off += n
            twt = {}
            for l in range(IN_LEVELS, TREE_DEPTH):
                m = 1 << (15 - l)
                a = wtile[:, off:off + m]; off += m
                b = wtile[:, off:off + m]; off += m
                twt[l] = (a, b)

            staging = stp.tile([P, ROWS, 1], dt.float16, tag="staging")

            for g in range(G):
                s = spool.tile([P, R, SUB], dt.float16, tag="s0")
                nc.sync.dma_start(out=s[:], in_=xin[:, g * R:(g + 1) * R, :])
                for l in range(IN_LEVELS):
                    n = SUB >> l
                    half = n // 2
                    pl = early if l < 2 else deep
                    z = pl.tile([P, R, n], dt.float16, tag=f"z{l}")
                    wb = (wtiles[l]
                          .rearrange("p (o n) -> p o n", o=1)
                          .broadcast_to([P, R, n]))
                    nc.vector.tensor_mul(z[:], s[:], wb)
                    adder = nc.gpsimd if l < GPSIMD_ADD_LEVELS else nc.vector
                    if l < IN_LEVELS - 1:
                        v = pl.tile([P, R, half], dt.float16, tag=f"v{l}")
                        adder.tensor_add(v[:], z[:, :, :half], z[:, :, half:])
                        sn = pl.tile([P, R, half], dt.float16, tag=f"s{l + 1}")
                        nc.scalar.activation(sn[:], v[:], relu_t)
                        s = sn
                    else:
                        # level 8: half == 1 -> into staging (with relu)
                        v = deep.tile([P, R, 1], dt.float16, tag="v8")
                        adder.tensor_add(v[:], z[:, :, 0:1], z[:, :, 1:2])
                        nc.scalar.activation(
                            staging[:, g * R:(g + 1) * R, :], v[:], relu_t)

            # ---- tail: levels 9..15 ----
            st2 = staging[:].rearrange("p r one -> p (r one)")  # [128, 256] fp16
            tt = []
            for hblk in range(2):
                tb = stp.tile([P, P], dt.float16, tag=f"tr{hblk}")
                nc.sync.dma_start_transpose(
                    out=tb[:], in_=st2[:, hblk * P:(hblk + 1) * P])
                tt.append(tb)

            ofin = stp.tile([P, 2], dt.float32, tag="ofin")
            for hblk in range(2):
                cur = tt[hblk]  # [128 rows, 128 nodes] fp16
                width = P
                for l in range(IN_LEVELS, TREE_DEPTH):
                    m = width // 2
                    a, b = twt[l]
                    z0 = stp.tile([P, m], dt.float16, tag=f"tz0_{l}_{hblk}")
                    z1 = stp.tile([P, m], dt.float16, tag=f"tz1_{l}_{hblk}")
                    nc.vector.tensor_mul(z0[:], cur[:, 0:width:2], a)
                    nc.vector.tensor_mul(z1[:], cur[:, 1:width:2], b)
                    if l < TREE_DEPTH - 1:
                        vv = stp.tile([P, m], dt.float16, tag=f"tv_{l}_{hblk}")
                        nc.vector.tensor_add(vv[:], z0[:], z1[:])
                        nxt = stp.tile([P, m], dt.float16, tag=f"ts_{l}_{hblk}")
                        nc.vector.tensor_scalar_max(nxt[:], vv[:], 0.0)
                        cur = nxt
                        width = m
                    else:
                        nc.vector.tensor_add(
                            ofin[:, hblk:hblk + 1], z0[:], z1[:])

            # out[r] for r in 0..127 from ofin[:,0]; 128..255 from ofin[:,1]
            for hblk in range(2):
                nc.sync.dma_start(
                    out=out[hblk * P:(hblk + 1) * P, :],
                    in_=ofin[:, hblk:hblk + 1])

    nc.compile()
    _CACHED["nc"] = nc
    return nc


def kernel(x, weights):
    from concourse.bass_utils import run_bass_kernel_spmd

    per_core_x, wparams = _host_pack(x, weights)
    nc = _build()
    in_maps = []
    for c in range(NCORES):
        m = {"x": per_core_x[c]}
        m.update(wparams)
        in_maps.append(m)
    res = run_bass_kernel_spmd(nc, in_maps, list(range(NCORES)))
    outs = [res.results[c]["out"] for c in range(NCORES)]
    return np.concatenate(outs, axis=0).astype(np.float32)


# revision 9
# speedup vs baseline: 1.5078x; 1.5078x over previous
"""Trainium2 Bass kernel for nn_LocalNet (binary-tree reduction network).

Computation: x [2048, 65536] f32; 16 levels of per-pair Linear(2,1) + ReLU
(no ReLU on the last level) -> out [2048, 1].

Strategy (pure data parallel, 8 cores, 256 rows each):
- Host: within each 512-feature partition block, permute columns by 9-bit
  bit-reversal.  This makes every tree level "planar": pair partners sit at
  (i, i + n/2), so all DVE accesses are unit-stride and fp16 tensor_tensor
  runs in 2x mode.  x is pre-cast to fp16 on host (bit-identical to the
  on-chip ScalarE cast it replaces; compute is fp16 anyway) halving DMA.
- Device, per core: stream groups of R rows as [128, R, 512] fp16 tiles
  (partition p holds that row's features [512p, 512p+512) bitrev-permuted),
  then levels 0..8:
      z  = s * wcat             (VectorE; wcat = [W0 | W1] planar, row-bcast)
      v  = z[:half] + z[half:]  (GpSimd for levels 0-1, else VectorE)
      s' = relu(v)              (ScalarE)
  Level-8 outputs accumulate into a [128, 256] staging tile (node q of each
  row's 128-node level-9 input lives on partition q).  Two 128x128 DMA-xbar
  transposes flip rows onto partitions; levels 9..15 then run along the free
  axis with host-replicated weights.  Final [256,1] f32 DMAed out per core.
"""

import sys

for _p in ("/opt/trn_rl_repo",):
    if _p not in sys.path:
        sys.path.insert(0, _p)

import numpy as np

TREE_DEPTH = 16
BATCH = 2048
FEATS = 65536
NCORES = 8
ROWS = BATCH // NCORES      # 256 rows per core
P = 128                     # SBUF partitions
SUB = FEATS // P            # 512 features per partition subtree
R = 16                      # rows per streamed group
G = ROWS // R               # groups
IN_LEVELS = 9               # levels 0..8 run inside partitions
GPSIMD_ADD_LEVELS = 0       # adds for levels 0..k-1 go to GpSimd


def _bitrev_array(bits):
    n = 1 << bits
    r = np.zeros(n, dtype=np.int64)
    for i in range(n):
        v = 0
        for b in range(bits):
            if i & (1 << b):
                v |= 1 << (bits - 1 - b)
        r[i] = v
    return r


def _host_pack(x, weights):
    """Build per-core input arrays + shared weight arrays."""
    brev = _bitrev_array(9)
    xs = np.asarray(x, dtype=np.float32).reshape(BATCH, P, SUB)[:, :, brev]
    xs = xs.astype(np.float16)
    per_core_x = []
    for c in range(NCORES):
        xc = np.ascontiguousarray(xs[c * ROWS:(c + 1) * ROWS].transpose(1, 0, 2))
        per_core_x.append(xc)  # [128, 256, 512] fp16

    blocks = []
    for l in range(IN_LEVELS):
        n = SUB >> l
        half = n // 2
        idx = _bitrev_array(8 - l) if half > 1 else np.zeros(1, dtype=np.int64)
        W = np.asarray(weights[l])                      # [2^(15-l), 2]
        q = np.arange(P)[:, None]
        g = q * half + idx[None, :]
        W0 = W[g, 0].astype(np.float16)
        W1 = W[g, 1].astype(np.float16)
        blocks.append(np.concatenate([W0, W1], axis=1))  # [128, n] fp16
    for l in range(IN_LEVELS, TREE_DEPTH):
        m = 1 << (15 - l)
        W = np.asarray(weights[l]).astype(np.float16)    # [m, 2]
        blocks.append(np.broadcast_to(W[None, :, 0], (P, m)))
        blocks.append(np.broadcast_to(W[None, :, 1], (P, m)))
    wall = np.ascontiguousarray(np.concatenate(blocks, axis=1))  # [128, WTOT]
    wparams = {"wall": wall}
    return per_core_x, wparams


_CACHED = {}


def _build():
    """Build the (SPMD-identical) Bass module once."""
    if "nc" in _CACHED:
        return _CACHED["nc"]
    import concourse.bacc as bacc
    import concourse.mybir as mybir
    import concourse.tile as tile

    dt = mybir.dt
    nc = bacc.Bacc(
        "TRN2", target_bir_lowering=False, debug=False, num_devices=NCORES)

    xin = nc.declare_dram_parameter("x", [P, ROWS, SUB], dt.float16, isOutput=False)
    WTOT = sum(SUB >> l for l in range(IN_LEVELS)) + 2 * sum(
        1 << (15 - l) for l in range(IN_LEVELS, TREE_DEPTH))
    wallp = nc.declare_dram_parameter("wall", [P, WTOT], dt.float16, isOutput=False)
    out = nc.declare_dram_parameter("out", [ROWS, 1], dt.float32, isOutput=True)

    relu_t = mybir.ActivationFunctionType.Relu

    with tile.TileContext(nc) as tc:
        with (
            tc.tile_pool(name="sio", bufs=3) as spool,
            tc.tile_pool(name="early", bufs=2) as early,
            tc.tile_pool(name="deep", bufs=2) as deep,
            tc.tile_pool(name="weights", bufs=1) as wp,
            tc.tile_pool(name="stage", bufs=1) as stp,
        ):
            # Load all weights in ONE DMA; slice views per level.
            wtile = wp.tile([P, WTOT], dt.float16, tag="wall")
            nc.sync.dma_start(out=wtile[:], in_=wallp[:])
            wtiles = []
            off = 0
            for l in range(IN_LEVELS):
                n = SUB >> l
                wtiles.append(wtile[:, off:off + n])
                off += n
            twt = {}
            for l in range(IN_LEVELS, TREE_DEPTH):
                m = 1 << (15 - l)
                a = wtile[:, off:off + m]; off += m
                b = wtile[:, off:off + m]; off += m
                twt[l] = (a, b)

            staging = stp.tile([P, ROWS, 1], dt.float16, tag="staging")

            for g in range(G):
                s = spool.tile([P, R, SUB], dt.float16, tag="s0")
                nc.sync.dma_start(out=s[:], in_=xin[:, g * R:(g + 1) * R, :])
                for l in range(IN_LEVELS):
                    n = SUB >> l
                    half = n // 2
                    pl = early if l < 2 else deep
                    z = pl.tile([P, R, n], dt.float16, tag=f"z{l}")
                    wb = (wtiles[l]
                          .rearrange("p (o n) -> p o n", o=1)
                          .broadcast_to([P, R, n]))
                    nc.vector.tensor_mul(z[:], s[:], wb)
                    adder = nc.gpsimd if l < GPSIMD_ADD_LEVELS else nc.vector
                    if l < IN_LEVELS - 1:
                        v = pl.tile([P, R, half], dt.float16, tag=f"v{l}")
                        adder.tensor_add(v[:], z[:, :, :half], z[:, :, half:])
                        sn = pl.tile([P, R, half], dt.float16, tag=f"s{l + 1}")
                        nc.scalar.activation(sn[:], v[:], relu_t)
                        s = sn
                    else:
                        # level 8: half == 1 -> into staging (with relu)
                        v = deep.tile([P, R, 1], dt.float16, tag="v8")
                        adder.tensor_add(v[:], z[:, :, 0:1], z[:, :, 1:2])
                        nc.scalar.activation(
                            staging[:, g * R:(g + 1) * R, :], v[:], relu_t)

            # ---- tail: levels 9..15 ----
            st2 = staging[:].rearrange("p r one -> p (r one)")  # [128, 256] fp16
            tt = []
            for hblk in range(2):
                tb = stp.tile([P, P], dt.float16, tag=f"tr{hblk}")
                nc.sync.dma_start_transpose(
                    out=tb[:], in_=st2[:, hblk * P:(hblk + 1) * P])
                tt.append(tb)

            ofin = stp.tile([P, 2], dt.float32, tag="ofin")
            for hblk in range(2):
                cur = tt[hblk]  # [128 rows, 128 nodes] fp16
                width = P
                for l in range(IN_LEVELS, TREE_DEPTH):
                    m = width // 2
                    a, b = twt[l]
                    z0 = stp.tile([P, m], dt.float16, tag=f"tz0_{l}_{hblk}")
                    z1 = stp.tile([P, m], dt.float16, tag=f"tz1_{l}_{hblk}")
                    nc.vector.tensor_mul(z0[:], cur[:, 0:width:2], a)
                    nc.vector.tensor_mul(z1[:], cur[:, 1:width:2], b)
                    if l < TREE_DEPTH - 1:
                        vv = stp.tile([P, m], dt.float16, tag=f"tv_{l}_{hblk}")
                        nc.vector.tensor_add(vv[:], z0[:], z1[:])
                        nxt = stp.tile([P, m], dt.float16, tag=f"ts_{l}_{hblk}")
                        nc.vector.tensor_scalar_max(nxt[:], vv[:], 0.0)
                        cur = nxt
                        width = m
                    else:
                        nc.vector.tensor_add(
                            ofin[:, hblk:hblk + 1], z0[:], z1[:])

            # out[r] for r in 0..127 from ofin[:,0]; 128..255 from ofin[:,1]
            for hblk in range(2):
                nc.sync.dma_start(
                    out=out[hblk * P:(hblk + 1) * P, :],
                    in_=ofin[:, hblk:hblk + 1])

    nc.compile()
    _CACHED["nc"] = nc
    return nc


def kernel(x, weights):
    from concourse.bass_utils import run_bass_kernel_spmd

    per_core_x, wparams = _host_pack(x, weights)
    nc = _build()
    in_maps = []
    for c in range(NCORES):
        m = {"x": per_core_x[c]}
        m.update(wparams)
        in_maps.append(m)
    res = run_bass_kernel_spmd(nc, in_maps, list(range(NCORES)))
    outs = [res.results[c]["out"] for c in range(NCORES)]
    return np.concatenate(outs, axis=0).astype(np.float32)


# revision 13
# speedup vs baseline: 1.5534x; 1.0303x over previous
"""Trainium2 Bass kernel for nn_LocalNet (binary-tree reduction network).

Computation: x [2048, 65536] f32; 16 levels of per-pair Linear(2,1) + ReLU
(no ReLU on the last level) -> out [2048, 1].

Strategy (pure data parallel, 8 cores, 256 rows each):
- Host: within each 512-feature partition block, permute columns by 9-bit
  bit-reversal.  This makes every tree level "planar": pair partners sit at
  (i, i + n/2), so all DVE accesses are unit-stride and fp16 tensor_tensor
  runs in 2x mode.  x is pre-cast to fp16 on host (bit-identical to the
  on-chip ScalarE cast it replaces; compute is fp16 anyway) halving DMA.
- Device, per core: stream groups of R rows as [128, R, 512] fp16 tiles
  (partition p holds that row's features [512p, 512p+512) bitrev-permuted),
  then levels 0..8:
      z  = s * wcat             (VectorE; wcat = [W0 | W1] planar, row-bcast)
      v  = z[:half] + z[half:]  (GpSimd for levels 0-1, else VectorE)
      s' = relu(v)              (ScalarE)
  Level-8 outputs accumulate into a [128, 256] staging tile (node q of each
  row's 128-node level-9 input lives on partition q).  Two 128x128 DMA-xbar
  transposes flip rows onto partitions; levels 9..15 then run along the free
  axis with host-replicated weights.  Final [256,1] f32 DMAed out per core.
"""

import sys

for _p in ("/opt/trn_rl_repo",):
    if _p not in sys.path:
        sys.path.insert(0, _p)

import numpy as np

TREE_DEPTH = 16
BATCH = 2048
FEATS = 65536
NCORES = 8
ROWS = BATCH // NCORES      # 256 rows per core
P = 128                     # SBUF partitions
SUB = FEATS // P            # 512 features per partition subtree
R = 16                      # rows per streamed group
G = ROWS // R               # groups
IN_LEVELS = 9               # levels 0..8 run inside partitions
GPSIMD_ADD_LEVELS = 0       # adds for levels 0..k-1 go to GpSimd


def _bitrev_array(bits):
    n = 1 << bits
    r = np.zeros(n, dtype=np.int64)
    for i in range(n):
        v = 0
        for b in range(bits):
            if i & (1 << b):
                v |= 1 << (bits - 1 - b)
        r[i] = v
    return r


def _host_pack(x, weights):
    """Build per-core input arrays + shared weight arrays."""
    brev = _bitrev_array(9)
    xs = np.asarray(x, dtype=np.float32).reshape(BATCH, P, SUB)[:, :, brev]
    xs = xs.astype(np.float16)
    per_core_x = []
    for c in range(NCORES):
        xc = np.ascontiguousarray(xs[c * ROWS:(c + 1) * ROWS].transpose(1, 0, 2))
        per_core_x.append(xc)  # [128, 256, 512] fp16

    blocks = []
    for l in range(IN_LEVELS):
        n = SUB >> l
        half = n // 2
        idx = _bitrev_array(8 - l) if half > 1 else np.zeros(1, dtype=np.int64)
        W = np.asarray(weights[l])                      # [2^(15-l), 2]
        q = np.arange(P)[:, None]
        g = q * half + idx[None, :]
        W0 = W[g, 0].astype(np.float16)
        W1 = W[g, 1].astype(np.float16)
        blocks.append(np.concatenate([W0, W1], axis=1))  # [128, n] fp16
    for l in range(IN_LEVELS, TREE_DEPTH):
        m = 1 << (15 - l)
        W = np.asarray(weights[l]).astype(np.float16)    # [m, 2]
        blocks.append(np.broadcast_to(W[None, :, 0], (P, m)))
        blocks.append(np.broadcast_to(W[None, :, 1], (P, m)))
    wall = np.ascontiguousarray(np.concatenate(blocks, axis=1))  # [128, WTOT]
    wparams = {"wall": wall}
    return per_core_x, wparams


_CACHED = {}


def _build():
    """Build the (SPMD-identical) Bass module once."""
    if "nc" in _CACHED:
        return _CACHED["nc"]
    import concourse.bacc as bacc
    import concourse.mybir as mybir
    import concourse.tile as tile

    dt = mybir.dt
    nc = bacc.Bacc(
        "TRN2", target_bir_lowering=False, debug=False, num_devices=NCORES)

    xin = nc.declare_dram_parameter("x", [P, ROWS, SUB], dt.float16, isOutput=False)
    WTOT = sum(SUB >> l for l in range(IN_LEVELS)) + 2 * sum(
        1 << (15 - l) for l in range(IN_LEVELS, TREE_DEPTH))
    wallp = nc.declare_dram_parameter("wall", [P, WTOT], dt.float16, isOutput=False)
    out = nc.declare_dram_parameter("out", [ROWS, 1], dt.float32, isOutput=True)

    relu_t = mybir.ActivationFunctionType.Relu

    with tile.TileContext(nc) as tc:
        with (
            tc.tile_pool(name="sio", bufs=2) as spool,
            tc.tile_pool(name="early", bufs=2) as early,
            tc.tile_pool(name="deep", bufs=1) as deep,
            tc.tile_pool(name="weights", bufs=1) as wp,
            tc.tile_pool(name="stage", bufs=1) as stp,
        ):
            # Load all weights in ONE DMA; slice views per level.
            wtile = wp.tile([P, WTOT], dt.float16, tag="wall")
            nc.sync.dma_start(out=wtile[:], in_=wallp[:])
            wtiles = []
            off = 0
            for l in range(IN_LEVELS):
                n = SUB >> l
                wtiles.append(wtile[:, off:off + n])
                off += n
            twt = {}
            for l in range(IN_LEVELS, TREE_DEPTH):
                m = 1 << (15 - l)
                a = wtile[:, off:off + m]; off += m
                b = wtile[:, off:off + m]; off += m
                twt[l] = (a, b)

            EARLY_LEVELS = 3           # levels 0..2 per group
            HROWS = ROWS // 2          # 128 rows per half
            DW = SUB >> EARLY_LEVELS   # 64: level-3 input width per partition
            # level-3 inputs for one half-batch: [128, 128 rows, 64]
            s3h = [stp.tile([P, HROWS, DW], dt.float16, tag=f"s3h{h}",
                            name=f"s3h{h}")
                   for h in range(2)]
            ofin = stp.tile([P, 2], dt.float32, tag="ofin")

            def deep_and_tail(h):
                """Levels 3..8 batched over 64-row chunks, then transpose+tail."""
                CH = HROWS // 2
                stg = stp.tile([P, HROWS, 1], dt.float16, tag=f"stg{h}",
                               name=f"stg{h}")
                for ck in range(2):
                    s = s3h[h][:, ck * CH:(ck + 1) * CH, :]
                    for l in range(EARLY_LEVELS, IN_LEVELS):
                        n = SUB >> l
                        half = n // 2
                        z = deep.tile([P, CH, n], dt.float16, tag=f"dz{l}")
                        wb = (wtiles[l]
                              .rearrange("p (o n) -> p o n", o=1)
                              .broadcast_to([P, CH, n]))
                        nc.vector.tensor_mul(z[:], s, wb)
                        v = deep.tile([P, CH, half], dt.float16, tag=f"dv{l}")
                        nc.vector.tensor_add(v[:], z[:, :, :half], z[:, :, half:])
                        if l < IN_LEVELS - 1:
                            sn = deep.tile([P, CH, half], dt.float16, tag=f"ds{l+1}")
                            nc.scalar.activation(sn[:], v[:], relu_t)
                            s = sn[:]
                        else:
                            nc.scalar.activation(
                                stg[:, ck * CH:(ck + 1) * CH, :], v[:], relu_t)
                # transpose [128, 128] -> rows on partitions
                tb = stp.tile([P, P], dt.float16, tag=f"tr{h}", name=f"tr{h}")
                nc.sync.dma_start_transpose(
                    out=tb[:], in_=stg[:].rearrange("p r one -> p (r one)"))
                cur = tb
                width = P
                for l in range(IN_LEVELS, TREE_DEPTH):
                    m = width // 2
                    a, b = twt[l]
                    z0 = stp.tile([P, m], dt.float16, tag=f"tz0_{l}_{h}")
                    z1 = stp.tile([P, m], dt.float16, tag=f"tz1_{l}_{h}")
                    nc.vector.tensor_mul(z0[:], cur[:, 0:width:2], a)
                    nc.vector.tensor_mul(z1[:], cur[:, 1:width:2], b)
                    if l < TREE_DEPTH - 1:
                        vv = stp.tile([P, m], dt.float16, tag=f"tv_{l}_{h}")
                        nc.vector.tensor_add(vv[:], z0[:], z1[:])
                        nxt = stp.tile([P, m], dt.float16, tag=f"ts_{l}_{h}")
                        nc.vector.tensor_scalar_max(nxt[:], vv[:], 0.0)
                        cur = nxt
                        width = m
                    else:
                        nc.vector.tensor_add(
                            ofin[:, h:h + 1], z0[:], z1[:])

            GH = G // 2  # groups per half
            for g in range(G):
                h = g // GH
                s = spool.tile([P, R, SUB], dt.float16, tag="s0")
                nc.sync.dma_start(out=s[:], in_=xin[:, g * R:(g + 1) * R, :])
                for l in range(EARLY_LEVELS):
                    n = SUB >> l
                    half = n // 2
                    pl = early if l == 0 else deep
                    z = pl.tile([P, R, n], dt.float16, tag=f"z{l}")
                    wb = (wtiles[l]
                          .rearrange("p (o n) -> p o n", o=1)
                          .broadcast_to([P, R, n]))
                    nc.vector.tensor_mul(z[:], s[:], wb)
                    v = pl.tile([P, R, half], dt.float16, tag=f"v{l}")
                    nc.vector.tensor_add(v[:], z[:, :, :half], z[:, :, half:])
                    if l < EARLY_LEVELS - 1:
                        sn = pl.tile([P, R, half], dt.float16, tag=f"s{l + 1}")
                        nc.scalar.activation(sn[:], v[:], relu_t)
                        s = sn
                    else:
                        # write level-3 inputs into the half staging tile
                        gi = g % GH
                        nc.scalar.activation(
                            s3h[h][:, gi * R:(gi + 1) * R, :], v[:], relu_t)
                if g % GH == GH - 1:
                    deep_and_tail(h)

            # out[r] for r in 0..127 from ofin[:,0]; 128..255 from ofin[:,1]
            for hblk in range(2):
                nc.sync.dma_start(
                    out=out[hblk * P:(hblk + 1) * P, :],
                    in_=ofin[:, hblk:hblk + 1])

    nc.compile()
    _CACHED["nc"] = nc
    return nc


def kernel(x, weights):
    from concourse.bass_utils import run_bass_kernel_spmd

    per_core_x, wparams = _host_pack(x, weights)
    nc = _build()
    in_maps = []
    for c in range(NCORES):
        m = {"x": per_core_x[c]}
        m.update(wparams)
        in_maps.append(m)
    res = run_bass_kernel_spmd(nc, in_maps, list(range(NCORES)))
    outs = [res.results[c]["out"] for c in range(NCORES)]
    return np.concatenate(outs, axis=0).astype(np.float32)
